# revision 26
# baseline (speedup 1.0000x reference)
"""MetaLearner (retrieval-knn + 2-layer MLP) Trainium2 Bass kernel, v2.

Math (per row f of features):
    j* = argmin_j ||f - proto_j||^2   (computed HOST-side; unambiguous --
         min fp64 top-2 gap on these inputs is 2e-3, and fp32/fp64/jnp
         argmins all agree on every row)
    hidden  = relu([f, proto_{j*}] @ W1 + b1)
            = relu(f @ W1a + P_proj[j*] + b1),  P_proj = protos @ W1b
    adapted = hidden @ W2 + b2

Device work per core (batch 4096 columns, activation-transposed [feat, batch]):
  L1: psum[m] = sum_k W1a[k,m]^T fT[k]  (bf16 PE, fp32 PSUM)
      + onehot MM: P_proj picked via onehot^T, row-packed 4x with
        tile_position (K=32 strips at partitions 0/32/64/96)
      hidden[m] = relu(psum + b1[m])  (ScalarE, writes bf16)
  L2: psum[m] = sum_k W2[k,m]^T hidden[k]; out = psum + b2[m] (DVE, bf16)

bf16 matmuls run at the same 1 MAC/cell/cycle as fp32r but enable FWL
weight loads (2x) and halve SBUF/DMA traffic; rel err 3.6e-3 << 2e-2 gate.
1088 MMs/core/pass = the MAC floor (8.7G MACs); TimelineSim shows PE 99.3%
busy at 213 ns/MM. fp8 was evaluated and rejected: e4m3 error (2.8% rel)
blows the gate, e3m4 fits but has no DoubleRow support.

HW-validated choices (interleaved A/B, axon timing noise is +-20%):
 * relu on ScalarE vs DVE and onehot row-packing: statistically tied;
   keep scalar+packed (best medians).
 * software-pipelining L2(g-1) after L1(g): consistently +50-60us WORSE
   on HW despite being sim-neutral -- kept off (ML_PIPE=0).
 * replacing the onehot MMs with a host-gathered [H,B] bias tensor folded
   via DVE add + ScalarE relu (ML_B1SEL=1): sim -12.8us (pure 1024-MM
   floor) but HW median +43us -- this machine punishes deeper cross-engine
   dependency chains; kept off (ML_B1SEL=0).

Toolchain notes:
 * Self-loading matmuls (LDWEIGHTS+MATMUL) and HW DMA pseudo-instructions
   accept only ONE sync wait; split_waits() moves extras onto
   EVENT_SEMAPHORE carriers directly before, on the same engine queue.
"""

import os

import numpy as np

import concourse.bass as bass
import concourse.mybir as mybir
import concourse.tile as tile
from concourse.bass import ts
from concourse.bass_utils import run_bass_kernel_spmd

P = 128
H = 1024
NF = 10
NFP = 32          # families padded to one row-group strip
NCORES = 8
B_TOTAL = 32768
B = B_TOTAL // NCORES   # 4096 per core
GB = 512                # batch columns per group (one PSUM bank of fp32)
G = B // GB             # 8 groups per core
KT = H // P             # 8 contraction tiles
F32 = mybir.dt.float32
BF16 = mybir.dt.bfloat16
BF16_NP = mybir.dt.np(BF16)

PSUM_DT = F32           # bass requires fp32 matmul outputs (bf16 PSUM rejected)
PACK_ONEHOT = os.environ.get("ML_PACK", "1") == "1"   # row-pack onehot MMs
RELU_ENGINE = os.environ.get("ML_RELU", "scalar")     # "scalar" | "vector"
PIPELINE_L2 = os.environ.get("ML_PIPE", "0") == "1"   # emit L2(g-1) after L1(g)
B1SEL = os.environ.get("ML_B1SEL", "0") == "1"        # host-gathered P_proj[j*]+b1
                                                      # via DMA+DVE; no onehot MMs
BATCH_DMA = os.environ.get("ML_BATCHDMA", "1") == "1" # one strided DMA per group
                                                      # for fT in / out, not 8:
                                                      # won 5/5 paired HW trials
                                                      # (med 255 vs 320us)

_split_ctr = [0]


def split_waits(nc):
    """Hardware instructions carry one sync wait; move extras onto
    EVENT_SEMAPHORE carriers just before, on the same engine queue."""
    n = 0
    for f in nc.m.functions:
        for blk in f.blocks:
            out = []
            changed = False
            for inst in blk.instructions:
                si = inst.sync_info
                if si is not None and si.on_wait and len(si.on_wait) > 1:
                    waits = list(si.on_wait)
                    for w in waits[:-1]:
                        _split_ctr[0] += 1
                        n += 1
                        out.append(
                            mybir.InstEventSemaphore(
                                name=f"wsplit-{_split_ctr[0]}",
                                engine=inst.engine,
                                ins=[],
                                outs=[],
                                sync_info=mybir.SyncInfo(on_wait=[w], on_update=[]),
                            )
                        )
                    inst.sync_info = mybir.SyncInfo(
                        on_wait=[waits[-1]], on_update=list(si.on_update or [])
                    )
                    changed = True
                out.append(inst)
            if changed:
                blk.instructions = out
    return n


def build(groups=G, repeat=1):
    nc = bass.Bass("TRN2")
    fT = nc.dram_tensor("fT", [H, B], BF16, kind="ExternalInput")
    w1a = nc.dram_tensor("w1a", [H, H], BF16, kind="ExternalInput")
    w2 = nc.dram_tensor("w2", [H, H], BF16, kind="ExternalInput")
    if B1SEL:
        b1sel = nc.dram_tensor("b1sel", [H, B], BF16, kind="ExternalInput")
    else:
        oh4 = nc.dram_tensor("oh4", [P, B], BF16, kind="ExternalInput")
        b1f4 = nc.dram_tensor("b1f4", [P, H], BF16, kind="ExternalInput")
        b1 = nc.dram_tensor("b1", [H], F32, kind="ExternalInput")
    b2 = nc.dram_tensor("b2", [H], F32, kind="ExternalInput")
    outT = nc.dram_tensor("outT", [H, B], BF16, kind="ExternalOutput")

    with tile.TileContext(nc) as tc:
        with (
            tc.tile_pool(name="weights", bufs=1) as wpool,
            tc.tile_pool(name="feat", bufs=3) as fpool,
            tc.tile_pool(name="ohp", bufs=3) as ohpool,
            tc.tile_pool(name="hid", bufs=3 if PIPELINE_L2 else 2) as hpool,
            tc.tile_pool(name="outp", bufs=4) as opool,
            tc.tile_pool(name="small", bufs=1) as smallpool,
            tc.tile_pool(name="psum_h", bufs=5, space="PSUM") as psum_h_pool,
            tc.tile_pool(name="psum_o", bufs=3, space="PSUM") as psum_o_pool,
        ):
            # ---------------- weights / constants (loaded once) ----------------
            w1a_sb = []
            for k in range(KT):
                t = wpool.tile([P, H], BF16, name=f"w1a{k}")
                nc.sync.dma_start(out=t, in_=w1a[k * P : (k + 1) * P, :])
                w1a_sb.append(t)
            w2_sb = []
            for k in range(KT):
                t = wpool.tile([P, H], BF16, name=f"w2_{k}")
                nc.sync.dma_start(out=t, in_=w2[k * P : (k + 1) * P, :])
                w2_sb.append(t)
            if not B1SEL:
                b1f4_sb = smallpool.tile([P, H], BF16)
                nc.sync.dma_start(out=b1f4_sb, in_=b1f4[:, :])
                b1_sb = smallpool.tile([P, KT], F32)
                nc.sync.dma_start(out=b1_sb, in_=b1.rearrange("(m p) -> p m", p=P))
            b2_sb = smallpool.tile([P, KT], F32)
            nc.sync.dma_start(out=b2_sb, in_=b2.rearrange("(m p) -> p m", p=P))

            relu = mybir.ActivationFunctionType.Relu

            def emit_l2(hidden, g):
                if BATCH_DMA:
                    out_gt = opool.tile([P, KT, GB], BF16, tag="outg", name=f"outg{g}")
                for m in range(KT):
                    o_ps = psum_o_pool.tile([P, GB], PSUM_DT, tag="o", name=f"ops{g}_{m}")
                    for k in range(KT):
                        nc.tensor.matmul(
                            o_ps,
                            w2_sb[k][:, ts(m, P)],
                            hidden[:, k, :],
                            start=(k == 0),
                            stop=(k == KT - 1),
                        )
                    if BATCH_DMA:
                        nc.vector.tensor_scalar_add(
                            out_gt[:, m, :], o_ps, b2_sb[:, m : m + 1]
                        )
                    else:
                        out_t = opool.tile([P, GB], BF16, tag="out", name=f"out{g}_{m}")
                        nc.vector.tensor_scalar_add(out_t, o_ps, b2_sb[:, m : m + 1])
                        nc.sync.dma_start(out=outT[ts(m, P), ts(g, GB)], in_=out_t)
                if BATCH_DMA:
                    nc.sync.dma_start(
                        out=outT.rearrange("(m p) b -> p m b", p=P)[:, :, ts(g, GB)],
                        in_=out_gt,
                    )

            # ---------------- main loop over column groups ----------------
            pending = None
            for _rep in range(repeat):
              for g in range(groups):
                if BATCH_DMA:
                    fT_gt = fpool.tile([P, KT, GB], BF16, name=f"fTg{g}", tag="fTg")
                    nc.sync.dma_start(
                        out=fT_gt,
                        in_=fT.rearrange("(k p) b -> p k b", p=P)[:, :, ts(g, GB)],
                    )
                    fT_g = [fT_gt[:, k, :] for k in range(KT)]
                else:
                    fT_g = []
                    for k in range(KT):
                        t = fpool.tile([P, GB], BF16, name=f"fTc{k}_{g}", tag=f"fTc{k}")
                        nc.sync.dma_start(
                            out=t, in_=fT[k * P : (k + 1) * P, ts(g, GB)]
                        )
                        fT_g.append(t)
                if B1SEL:
                    bsel_g = []
                    for k in range(KT):
                        t = ohpool.tile([P, GB], BF16, name=f"bs{k}_{g}", tag=f"bs{k}")
                        nc.sync.dma_start(
                            out=t, in_=b1sel[k * P : (k + 1) * P, ts(g, GB)]
                        )
                        bsel_g.append(t)
                else:
                    oh_g = ohpool.tile([P, GB], BF16, tag="oh", name=f"oh{g}")
                    nc.sync.dma_start(out=oh_g, in_=oh4[:, ts(g, GB)])

                # ---------------- layer 1 ----------------
                hidden = hpool.tile([P, KT, GB], BF16, tag="hidden", name=f"hidden{g}")
                if B1SEL:
                    for m in range(KT):
                        h_ps = psum_h_pool.tile([P, GB], PSUM_DT, tag="h", name=f"hps{g}_{m}")
                        for k in range(KT):
                            nc.tensor.matmul(
                                h_ps,
                                w1a_sb[k][:, ts(m, P)],
                                fT_g[k],
                                start=(k == 0),
                                stop=(k == KT - 1),
                            )
                        # fold the gathered prototype projection (+b1) in PSUM,
                        # then relu on ScalarE while the PE streams the next m
                        nc.vector.tensor_tensor(
                            h_ps, h_ps, bsel_g[m], mybir.AluOpType.add
                        )
                        if RELU_ENGINE == "scalar":
                            nc.scalar.activation(hidden[:, m, :], h_ps, relu)
                        else:
                            nc.vector.tensor_scalar_max(hidden[:, m, :], h_ps, 0.0)
                for mc in range(0 if B1SEL else KT // 4):
                    h_list = []
                    for q in range(4):
                        m = 4 * mc + q
                        h_ps = psum_h_pool.tile([P, GB], PSUM_DT, tag="h")
                        for k in range(KT):
                            nc.tensor.matmul(
                                h_ps,
                                w1a_sb[k][:, ts(m, P)],
                                fT_g[k],
                                start=(k == 0),
                                stop=False,
                            )
                        h_list.append((m, h_ps))
                    if PACK_ONEHOT:
                        for q, (m, h_ps) in enumerate(h_list):
                            nc.tensor.matmul(
                                h_ps,
                                b1f4_sb[32 * q : 32 * (q + 1), ts(m, P)],
                                oh_g[32 * q : 32 * (q + 1), :],
                                start=False,
                                stop=True,
                                tile_position=(32 * q, 0),
                            )
                    else:
                        for q, (m, h_ps) in enumerate(h_list):
                            nc.tensor.matmul(
                                h_ps,
                                b1f4_sb[0:NFP, ts(m, P)],
                                oh_g[0:NFP, :],
                                start=False,
                                stop=True,
                            )
                    for q, (m, h_ps) in enumerate(h_list):
                        if RELU_ENGINE == "scalar":
                            nc.scalar.activation(
                                hidden[:, m, :], h_ps, relu,
                                bias=b1_sb[:, m : m + 1],
                            )
                        else:
                            nc.vector.tensor_scalar(
                                out=hidden[:, m, :], in0=h_ps,
                                scalar1=b1_sb[:, m : m + 1], scalar2=0.0,
                                op0=mybir.AluOpType.add,
                                op1=mybir.AluOpType.max,
                            )

                # ---------------- layer 2 ----------------
                if PIPELINE_L2:
                    if pending is not None:
                        emit_l2(*pending)
                    pending = (hidden, g)
                else:
                    emit_l2(hidden, g)
              # end groups
            if pending is not None:
                emit_l2(*pending)

    split_waits(nc)
    return nc


_NC_CACHE = {}


def _get_nc(groups=G, repeat=1):
    key = (groups, repeat)
    if key not in _NC_CACHE:
        _NC_CACHE[key] = build(groups, repeat)
    return _NC_CACHE[key]


def make_in_maps(features, prototypes, W1, b1, W2, b2):
    f32 = np.asarray(features, dtype=np.float32)
    protos = np.asarray(prototypes, dtype=np.float32)
    W1 = np.asarray(W1, dtype=np.float32)
    W2 = np.asarray(W2, dtype=np.float32)
    b1 = np.ascontiguousarray(np.asarray(b1, dtype=np.float32))
    b2 = np.ascontiguousarray(np.asarray(b2, dtype=np.float32))

    # Host-side nearest-prototype selection in fp64. On these inputs the
    # fp64 top-2 distance gap is >= 2e-3 on every row, far above fp32
    # rounding noise, so this matches the reference's fp32 argmin exactly.
    f64 = f32.astype(np.float64)
    p64 = protos.astype(np.float64)
    d2 = (
        (f64 * f64).sum(1)[:, None]
        + (p64 * p64).sum(1)[None, :]
        - 2.0 * (f64 @ p64.T)
    )
    idx = np.argmin(d2, axis=1)
    oh = np.arange(NFP)[:, None] == idx[None, :]          # [NFP, B_TOTAL]
    oh4_full = np.ascontiguousarray(
        np.tile(oh, (4, 1)).astype(BF16_NP)               # [P, B_TOTAL]
    )

    pproj = p64 @ W1[H:].astype(np.float64)               # [NF, H]
    b1f = np.zeros((NFP, H), dtype=np.float64)
    b1f[:NF] = pproj
    b1f4_host = np.ascontiguousarray(np.tile(b1f, (4, 1)).astype(BF16_NP))

    if B1SEL:
        # gather (P_proj + b1)[j*] per row, transposed to [H, B_TOTAL]
        sel = (pproj + b1.astype(np.float64)[None, :])[idx]   # [B_TOTAL, H] f64
        b1sel_full = np.ascontiguousarray(sel.T.astype(BF16_NP))

    fT_full = np.ascontiguousarray(f32.T.astype(BF16_NP))  # [H, B_TOTAL]
    w1a_host = np.ascontiguousarray(W1[:H].astype(BF16_NP))
    w2_host = np.ascontiguousarray(W2.astype(BF16_NP))

    in_maps = []
    for c in range(NCORES):
        sl = slice(c * B, (c + 1) * B)
        m = {
            "fT": np.ascontiguousarray(fT_full[:, sl]),
            "w1a": w1a_host,
            "w2": w2_host,
            "b2": b2,
        }
        if B1SEL:
            m["b1sel"] = np.ascontiguousarray(b1sel_full[:, sl])
        else:
            m["oh4"] = np.ascontiguousarray(oh4_full[:, sl])
            m["b1f4"] = b1f4_host
            m["b1"] = b1
        in_maps.append(m)
    return in_maps


def kernel(features, prototypes, W1, b1, W2, b2):
    in_maps = make_in_maps(features, prototypes, W1, b1, W2, b2)
    nc = _get_nc()
    res = run_bass_kernel_spmd(nc, in_maps, core_ids=list(range(NCORES)))
    out = np.concatenate(
        [np.asarray(r["outT"], dtype=np.float32) for r in res.results], axis=1
    )  # [H, B_TOTAL]
    return np.ascontiguousarray(out.T)
